# revision 4
# baseline (speedup 1.0000x reference)
"""Trainium2 Bass kernel for per-sample modulated+demodulated 3D conv.

Problem: x[B=8, CIN=128, 32,32,32], y[8,128], weight[128,128,3,3,3] (shared).
  w_b = weight * (1 + y[b,i]);  w_b *= rsqrt(sum_{i,k} w_b^2 + eps)  (per out-ch)
  out[b] = conv3d(x[b], w_b, same padding)

Sharding: data-parallel over batch, one sample per NeuronCore (8 cores).
Per core: 27 shift-matmuls ([CIN=128] x [COUT=128] stationary against an
H/W-zero-padded x volume) accumulate each PSUM tile; demodulation is folded
into the PSUM->SBUF evacuation as a per-partition scale. Matmul operands are
rounded to float32r (TF32-like, full PE rate).
"""

import sys

import numpy as np

try:
    import concourse.bass as bass
except ImportError:  # fresh grading dir: fall back to the repo checkout
    sys.path.insert(0, "/opt/trn_rl_repo")
    import concourse.bass as bass

import concourse.tile as tile
from concourse import bacc, mybir
from concourse.bass_utils import run_bass_kernel_spmd
from concourse.masks import make_identity

B, CIN, COUT, K = 8, 128, 128, 3
D = H = W = 32
T = K * K * K  # 27
HP = H + 2  # padded rows
WP = W + 2  # padded cols
EPS = 1e-8
N_CORES = 8

FP32 = mybir.dt.float32
MM_DT = mybir.dt.float32r  # matmul operand dtype (TF32-like, 1 cyc/row)

_CACHE = {}


def _build_program():
    nc = bacc.Bacc()
    xv = nc.dram_tensor("x", [CIN, D, H, W], FP32, kind="ExternalInput")
    yv = nc.dram_tensor("y", [CIN, 1], FP32, kind="ExternalInput")
    wv = nc.dram_tensor("w", [COUT, CIN, T], FP32, kind="ExternalInput")
    ov = nc.dram_tensor("out", [COUT, D, H * W], FP32, kind="ExternalOutput")

    PLANES_PER_CHUNK = 4
    N_CHUNKS = D // PLANES_PER_CHUNK

    with tile.TileContext(nc) as tc:
        with (
            tc.tile_pool(name="const", bufs=1) as const,
            tc.tile_pool(name="xnat", bufs=2) as xnat,
            tc.tile_pool(name="evac", bufs=4) as evac,
            tc.tile_pool(name="psum", bufs=4, space="PSUM") as psum,
            tc.tile_pool(name="psw", bufs=2, space="PSUM") as psw,
        ):
            # ---------------- weight prep ----------------
            wa = const.tile([COUT, CIN, T], FP32, tag="wa")
            nc.sync.dma_start(out=wa, in_=wv[:, :, :])
            ym = const.tile([CIN, 1], FP32, tag="ym")
            nc.sync.dma_start(out=ym, in_=yv[:, :])
            yrow = const.tile([1, CIN], FP32, tag="yrow")
            nc.sync.dma_start(out=yrow, in_=yv[:, 0])

            # (1 + y[i]) as a per-partition column (for modulation)
            ymp1 = const.tile([CIN, 1], FP32, tag="ymp1")
            nc.vector.tensor_scalar_add(ymp1, ym, 1.0)

            ident = const.tile([128, 128], FP32, tag="ident")
            make_identity(nc, ident)

            # transposed modulated weights: wT3[i, t, o] = w[o,i,t]*(1+y[i])
            wT3 = const.tile([CIN, T, COUT], MM_DT, tag="wT3")
            for t in range(T):
                pst = psw.tile([128, 128], FP32, tag="psw")
                nc.tensor.transpose(pst, wa[:, :, t], ident)
                nc.scalar.activation(
                    out=wT3[:, t, :],
                    in_=pst,
                    func=mybir.ActivationFunctionType.Copy,
                    scale=ymp1,
                )

            # demod scale: div[o] = rsqrt(sum_i s[i]*q[o,i] + eps),
            #   q[o,i] = sum_t w[o,i,t]^2,  s[i] = (1+y[i])^2
            wsq = const.tile([COUT, CIN, T], FP32, tag="wsq")
            nc.vector.tensor_mul(wsq, wa, wa)
            q = const.tile([COUT, CIN], FP32, tag="q")
            nc.vector.reduce_sum(q, wsq, axis=mybir.AxisListType.X)

            yp1row = const.tile([1, CIN], FP32, tag="yp1row")
            nc.vector.tensor_scalar_add(yp1row, yrow, 1.0)
            srow = const.tile([1, CIN], FP32, tag="srow")
            nc.vector.tensor_mul(srow, yp1row, yp1row)
            ones1 = const.tile([1, 128], FP32, tag="ones1")
            nc.vector.memset(ones1, 1.0)
            # broadcast srow across partitions via a K=1 matmul
            ps_s = psw.tile([128, 128], FP32, tag="psw")
            nc.tensor.matmul(ps_s, ones1, srow, start=True, stop=True)

            tq = const.tile([COUT, CIN], FP32, tag="tq")
            nc.vector.tensor_mul(tq, q, ps_s)
            ncol = const.tile([COUT, 1], FP32, tag="ncol")
            nc.vector.reduce_sum(ncol, tq, axis=mybir.AxisListType.X)
            epst = const.tile([COUT, 1], FP32, tag="epst")
            nc.vector.memset(epst, EPS)
            sqn = const.tile([COUT, 1], FP32, tag="sqn")
            nc.scalar.activation(
                out=sqn,
                in_=ncol,
                func=mybir.ActivationFunctionType.Sqrt,
                bias=epst,
                scale=1.0,
            )
            div = const.tile([COUT, 1], FP32, tag="div")
            nc.vector.reciprocal(div, sqn)

            # ---------------- x staging ----------------
            # ROT persistent padded-plane tiles; borders zeroed once (an
            # fp32->fp32r rounding copy, which the BIR verifier requires for
            # fp32r matmul operands) and never rewritten — per-plane repack
            # only touches the interior.
            ROT = 8
            ztile = const.tile([CIN, HP, WP], FP32, tag="ztile")
            nc.vector.memset(ztile, 0.0)
            pads = []
            for r in range(ROT):
                pt = const.tile([CIN, HP, WP], MM_DT, tag=f"pad{r}")
                nc.vector.tensor_copy(pt, ztile)
                pads.append(pt)

            pad_tiles = [None] * D
            nat_tiles = [None] * N_CHUNKS

            def stage_chunk(c):
                nat = xnat.tile([CIN, PLANES_PER_CHUNK, H, W], FP32, tag="xnat")
                p0 = c * PLANES_PER_CHUNK
                nc.sync.dma_start(out=nat, in_=xv[:, p0 : p0 + PLANES_PER_CHUNK, :, :])
                nat_tiles[c] = nat

            def stage_plane(p):
                c, pi = divmod(p, PLANES_PER_CHUNK)
                if nat_tiles[c] is None:
                    stage_chunk(c)
                pad = pads[p % ROT]
                nc.vector.tensor_copy(
                    pad[:, 1 : HP - 1, 1 : WP - 1], nat_tiles[c][:, pi, :, :]
                )
                pad_tiles[p] = pad

            # ---------------- conv main loop ----------------
            stage_plane(0)
            stage_plane(1)
            for d in range(D):
                if d + 1 < D and pad_tiles[d + 1] is None:
                    stage_plane(d + 1)
                # valid kernel-depth offsets for this output plane
                mms = []
                for kd in range(K):
                    dp = d + kd - 1
                    if dp < 0 or dp >= D:
                        continue
                    for kh in range(K):
                        for kw in range(K):
                            mms.append((kd * 9 + kh * 3 + kw, dp, kh, kw))
                for hp in range(2):
                    r0 = hp * 16
                    ps = psum.tile([COUT, 512], FP32, tag="psum")
                    for idx, (t, dp, kh, kw) in enumerate(mms):
                        rhs = pad_tiles[dp][:, r0 + kh : r0 + kh + 16, kw : kw + W]
                        nc.tensor.matmul(
                            ps,
                            wT3[:, t, :],
                            rhs,
                            start=(idx == 0),
                            stop=(idx == len(mms) - 1),
                        )
                    ot = evac.tile([COUT, 512], FP32, tag="evac")
                    nc.scalar.activation(
                        out=ot,
                        in_=ps,
                        func=mybir.ActivationFunctionType.Copy,
                        scale=div,
                    )
                    nc.sync.dma_start(
                        out=ov[:, d, r0 * W : r0 * W + 512], in_=ot
                    )

    nc.compile()
    return nc


def kernel(x: np.ndarray, y: np.ndarray, weight: np.ndarray) -> np.ndarray:
    x = np.ascontiguousarray(np.asarray(x, dtype=np.float32))
    y = np.ascontiguousarray(np.asarray(y, dtype=np.float32))
    weight = np.ascontiguousarray(np.asarray(weight, dtype=np.float32))

    if "nc" not in _CACHE:
        _CACHE["nc"] = _build_program()
    nc = _CACHE["nc"]

    wflat = weight.reshape(COUT, CIN, T)
    in_maps = [
        {
            "x": x[b],
            "y": y[b].reshape(CIN, 1),
            "w": wflat,
        }
        for b in range(B)
    ]
    res = run_bass_kernel_spmd(nc, in_maps, list(range(N_CORES)))
    out = np.stack(
        [res.results[b]["out"].reshape(COUT, D, H, W) for b in range(B)], axis=0
    )
    return out


if __name__ == "__main__":
    rng = np.random.default_rng(0)
    x = rng.standard_normal((B, CIN, D, H, W), dtype=np.float32)
    y = rng.standard_normal((B, CIN), dtype=np.float32)
    w = rng.standard_normal((COUT, CIN, K, K, K), dtype=np.float32) * 0.017
    out = kernel(x=x, y=y, weight=w)
    print("out", out.shape, out.dtype, float(np.abs(out).max()))


# revision 7
# speedup vs baseline: 1.0941x; 1.0941x over previous
"""Trainium2 Bass kernel for per-sample modulated+demodulated 3D conv.

Problem: x[B=8, CIN=128, 32,32,32], y[8,128], weight[128,128,3,3,3] (shared).
  w_b = weight * (1 + y[b,i]);  w_b *= rsqrt(sum_{i,k} w_b^2 + eps)  (per out-ch)
  out[b] = conv3d(x[b], w_b, same padding)

Sharding: data-parallel over batch, one sample per NeuronCore (8 cores).
Per core: 27 shift-matmuls ([CIN=128] x [COUT=128] stationary against an
H/W-zero-padded x volume) accumulate each PSUM tile; demodulation is folded
into the PSUM->SBUF evacuation as a per-partition scale. Matmul operands are
fp16 (full PE rate, 10-bit mantissa; fp32 PSUM accumulation).
"""

import sys

import numpy as np

try:
    import concourse.bass as bass
except ImportError:  # fresh grading dir: fall back to the repo checkout
    sys.path.insert(0, "/opt/trn_rl_repo")
    import concourse.bass as bass

import concourse.tile as tile
from concourse import bacc, mybir
from concourse.masks import make_identity

B, CIN, COUT, K = 8, 128, 128, 3
D = H = W = 32
T = K * K * K  # 27
HP = H + 2  # padded rows
WP = W + 2  # padded cols
EPS = 1e-8
N_CORES = 8

FP32 = mybir.dt.float32
MM_DT = mybir.dt.float16  # matmul operand dtype (fp32 PSUM accumulate)

_CACHE = {}


def _build_program():
    nc = bacc.Bacc()
    xv = nc.dram_tensor("x", [CIN, D, H, W], FP32, kind="ExternalInput")
    yv = nc.dram_tensor("y", [CIN, 1], FP32, kind="ExternalInput")
    wv = nc.dram_tensor("w", [COUT, CIN, T], FP32, kind="ExternalInput")
    ov = nc.dram_tensor("out", [COUT, D, H * W], FP32, kind="ExternalOutput")

    PLANES_PER_CHUNK = 4
    N_CHUNKS = D // PLANES_PER_CHUNK

    with tile.TileContext(nc) as tc:
        with (
            tc.tile_pool(name="const", bufs=1) as const,
            tc.tile_pool(name="xnat", bufs=2) as xnat,
            tc.tile_pool(name="evac", bufs=4) as evac,
            tc.tile_pool(name="psum", bufs=4, space="PSUM") as psum,
            tc.tile_pool(name="psw", bufs=4, space="PSUM") as psw,
        ):
            # ---------------- x staging (issue first: overlaps weight prep) ----
            ROT = 8
            pads = []
            for r in range(ROT):
                pt = const.tile([CIN, HP, WP], MM_DT, tag=f"pad{r}", name=f"pad{r}")
                pads.append(pt)

            pad_tiles = [None] * D
            nat_tiles = [None] * N_CHUNKS

            def stage_chunk(c):
                nat = xnat.tile(
                    [CIN, PLANES_PER_CHUNK, H, W], FP32, tag="xnat", name="nat"
                )
                p0 = c * PLANES_PER_CHUNK
                nc.sync.dma_start(out=nat, in_=xv[:, p0 : p0 + PLANES_PER_CHUNK, :, :])
                nat_tiles[c] = nat

            def stage_plane(p):
                c, pi = divmod(p, PLANES_PER_CHUNK)
                if nat_tiles[c] is None:
                    stage_chunk(c)
                pad = pads[p % ROT]
                nc.vector.tensor_copy(
                    pad[:, 1 : HP - 1, 1 : WP - 1], nat_tiles[c][:, pi, :, :]
                )
                pad_tiles[p] = pad

            stage_chunk(0)  # x DMA in flight while weights prep

            # zero pad borders once (interior repack never touches them)
            ztile = const.tile([CIN, HP, WP], FP32, tag="ztile")
            nc.vector.memset(ztile, 0.0)
            for pt in pads:
                nc.vector.tensor_copy(pt, ztile)

            # ---------------- weight prep ----------------
            wa = const.tile([COUT, CIN, T], FP32, tag="wa")
            nc.sync.dma_start(out=wa, in_=wv[:, :, :])
            ym = const.tile([CIN, 1], FP32, tag="ym")
            nc.sync.dma_start(out=ym, in_=yv[:, :])
            yrow = const.tile([1, CIN], FP32, tag="yrow")
            nc.sync.dma_start(out=yrow, in_=yv[:, 0])

            # (1 + y[i]) as a per-partition column (for modulation)
            ymp1 = const.tile([CIN, 1], FP32, tag="ymp1")
            nc.vector.tensor_scalar_add(ymp1, ym, 1.0)

            ident = const.tile([128, 128], FP32, tag="ident")
            make_identity(nc, ident)

            # transposed modulated weights: wT3[i, t, o] = w[o,i,t]*(1+y[i])
            wT3 = const.tile([CIN, T, COUT], MM_DT, tag="wT3")
            for t in range(T):
                pst = psw.tile([128, 128], FP32, tag="psw", name="pst")
                nc.tensor.transpose(pst, wa[:, :, t], ident)
                nc.scalar.activation(
                    out=wT3[:, t, :],
                    in_=pst,
                    func=mybir.ActivationFunctionType.Copy,
                    scale=ymp1,
                )

            # demod scale: div[o] = rsqrt(sum_i s[i]*q[o,i] + eps),
            #   q[o,i] = sum_t w[o,i,t]^2,  s[i] = (1+y[i])^2
            wsq = const.tile([COUT, CIN, T], FP32, tag="wsq")
            nc.vector.tensor_mul(wsq, wa, wa)
            q = const.tile([COUT, CIN], FP32, tag="q")
            nc.vector.reduce_sum(q, wsq, axis=mybir.AxisListType.X)

            yp1row = const.tile([1, CIN], FP32, tag="yp1row")
            nc.vector.tensor_scalar_add(yp1row, yrow, 1.0)
            srow = const.tile([1, CIN], FP32, tag="srow")
            nc.vector.tensor_mul(srow, yp1row, yp1row)
            ones1 = const.tile([1, 128], FP32, tag="ones1")
            nc.vector.memset(ones1, 1.0)
            # broadcast srow across partitions via a K=1 matmul
            ps_s = psw.tile([128, 128], FP32, tag="psw", name="ps_s")
            nc.tensor.matmul(ps_s, ones1, srow, start=True, stop=True)

            tq = const.tile([COUT, CIN], FP32, tag="tq")
            nc.vector.tensor_mul(tq, q, ps_s)
            ncol = const.tile([COUT, 1], FP32, tag="ncol")
            nc.vector.reduce_sum(ncol, tq, axis=mybir.AxisListType.X)
            epst = const.tile([COUT, 1], FP32, tag="epst")
            nc.vector.memset(epst, EPS)
            sqn = const.tile([COUT, 1], FP32, tag="sqn")
            nc.scalar.activation(
                out=sqn,
                in_=ncol,
                func=mybir.ActivationFunctionType.Sqrt,
                bias=epst,
                scale=1.0,
            )
            div = const.tile([COUT, 1], FP32, tag="div")
            nc.vector.reciprocal(div, sqn)

            # ---------------- conv main loop ----------------
            stage_plane(0)
            stage_plane(1)
            for d in range(D):
                if d + 1 < D and pad_tiles[d + 1] is None:
                    stage_plane(d + 1)
                # valid kernel-depth offsets for this output plane
                mms = []
                for kd in range(K):
                    dp = d + kd - 1
                    if dp < 0 or dp >= D:
                        continue
                    for kh in range(K):
                        for kw in range(K):
                            mms.append((kd * 9 + kh * 3 + kw, dp, kh, kw))
                for hp in range(2):
                    r0 = hp * 16
                    ps = psum.tile([COUT, 512], FP32, tag="psum", name="ps")
                    for idx, (t, dp, kh, kw) in enumerate(mms):
                        rhs = pad_tiles[dp][:, r0 + kh : r0 + kh + 16, kw : kw + W]
                        nc.tensor.matmul(
                            ps,
                            wT3[:, t, :],
                            rhs,
                            start=(idx == 0),
                            stop=(idx == len(mms) - 1),
                        )
                    ot = evac.tile([COUT, 512], FP32, tag="evac", name="ot")
                    nc.scalar.activation(
                        out=ot,
                        in_=ps,
                        func=mybir.ActivationFunctionType.Copy,
                        scale=div,
                    )
                    nc.sync.dma_start(out=ov[:, d, r0 * W : r0 * W + 512], in_=ot)

    nc.compile()
    return nc


def _make_runner(nc):
    """Build the jitted 8-core executor once (mirrors
    bass2jax.run_bass_via_pjrt's multi-core path, but cacheable)."""
    import jax
    from jax.experimental.shard_map import shard_map
    from jax.sharding import Mesh, PartitionSpec

    from concourse import bass2jax

    bass2jax.install_neuronx_cc_hook()

    partition_name = (
        nc.partition_id_tensor.name if nc.partition_id_tensor else None
    )
    in_names, out_names, out_avals, zero_shapes = [], [], [], []
    for alloc in nc.m.functions[0].allocations:
        if not isinstance(alloc, mybir.MemoryLocationSet):
            continue
        name = alloc.memorylocations[0].name
        if alloc.kind == "ExternalInput":
            if name != partition_name:
                in_names.append(name)
        elif alloc.kind == "ExternalOutput":
            out_names.append(name)
            shape = tuple(alloc.tensor_shape)
            dtype = mybir.dt.np(alloc.dtype)
            out_avals.append(jax.core.ShapedArray(shape, dtype))
            zero_shapes.append((shape, dtype))
    n_params = len(in_names)
    n_outs = len(out_names)
    bind_in_names = in_names + out_names
    if partition_name is not None:
        bind_in_names = bind_in_names + [partition_name]
    bind_in_names = tuple(bind_in_names)
    donate = tuple(range(n_params, n_params + n_outs))

    def _body(*args):
        operands = list(args)
        if partition_name is not None:
            operands.append(bass2jax.partition_id_tensor())
        outs = bass2jax._bass_exec_p.bind(
            *operands,
            out_avals=tuple(out_avals),
            in_names=bind_in_names,
            out_names=tuple(out_names),
            lowering_input_output_aliases=(),
            sim_require_finite=True,
            sim_require_nnan=True,
            nc=nc,
        )
        return tuple(outs)

    devices = jax.devices()[:N_CORES]
    mesh = Mesh(np.asarray(devices), ("core",))
    in_specs = (PartitionSpec("core"),) * (n_params + n_outs)
    out_specs = (PartitionSpec("core"),) * n_outs
    sharded = jax.jit(
        shard_map(
            _body, mesh=mesh, in_specs=in_specs, out_specs=out_specs, check_rep=False
        ),
        donate_argnums=donate,
        keep_unused=True,
    )

    def run(in_maps):
        concat_in = [
            np.concatenate([np.asarray(m[n]) for m in in_maps], axis=0)
            for n in in_names
        ]
        concat_zeros = [
            np.zeros((N_CORES * s[0], *s[1:]), dt) for s, dt in zero_shapes
        ]
        out_arrs = sharded(*concat_in, *concat_zeros)
        return [
            {
                n: np.asarray(out_arrs[i]).reshape(N_CORES, *out_avals[i].shape)[c]
                for i, n in enumerate(out_names)
            }
            for c in range(N_CORES)
        ]

    return run


def kernel(x: np.ndarray, y: np.ndarray, weight: np.ndarray) -> np.ndarray:
    x = np.ascontiguousarray(np.asarray(x, dtype=np.float32))
    y = np.ascontiguousarray(np.asarray(y, dtype=np.float32))
    weight = np.ascontiguousarray(np.asarray(weight, dtype=np.float32))

    if "run" not in _CACHE:
        _CACHE["nc"] = _build_program()
        _CACHE["run"] = _make_runner(_CACHE["nc"])
    run = _CACHE["run"]

    wflat = weight.reshape(COUT, CIN, T)
    in_maps = [
        {"x": x[b], "y": y[b].reshape(CIN, 1), "w": wflat} for b in range(B)
    ]
    results = run(in_maps)
    out = np.stack(
        [results[b]["out"].reshape(COUT, D, H, W) for b in range(B)], axis=0
    )
    return out


if __name__ == "__main__":
    rng = np.random.default_rng(0)
    x = rng.standard_normal((B, CIN, D, H, W), dtype=np.float32)
    y = rng.standard_normal((B, CIN), dtype=np.float32)
    w = rng.standard_normal((COUT, CIN, K, K, K), dtype=np.float32) * 0.017
    out = kernel(x=x, y=y, weight=w)
    print("out", out.shape, out.dtype, float(np.abs(out).max()))
